# revision 28
# baseline (speedup 1.0000x reference)
"""DTNNStep Bass kernel for Trainium2 (8 NeuronCores, data-parallel over batch).

Computes, per molecule b:
    dist_h = dist @ W_df + b_df              # [N, N, H]
    atom_h = atom @ W_cf + b_cf              # [N, H]
    gated  = dist_h * atom_h[None, :, :]     # broadcast over i
    out    = tanh((gated @ W_fc) * mask)     # mask == 1 in this benchmark
    result = out.sum(axis=1) + atom          # [N, F]

Per group of 8 output rows i (4 per partition half), software-pipelined in
3 stages so the in-order PE stream never waits on the PSUM drains:
  front(g): SWDGE fp32->bf16 dist window -> 8 PE transposes to [d, (u i j)]
            -> PSUM -> copy to SBUF dT (split scalar 704 / DVE 320 cols)
  back(g-2): mm2 pair -> scalar tanh -> DVE j-reduce (bf16, 2x) -> res_pack
  mid(g-1):  mm1 pair (K=100) -> DVE gate (out1 + b_df) * atom_h^T
Startup: all SWDGE descriptor gens issue up front (cold window in 4 pieces);
weights ride the sync HWDGE ring, atoms the scalar ring; only the bf16
identity is built before the gens.  PE holds ~1.2 GHz on this part
regardless of busy streaks, so the pipeline is sized to that clock.
"""

import os
import sys

import numpy as np

for _p in ("/opt/trn_rl_repo", os.path.expanduser("~/.axon_site/_ro/trn_rl_repo")):
    if os.path.isdir(_p) and _p not in sys.path:
        sys.path.insert(0, _p)

import concourse.bass as bass
import concourse.tile as tile
from concourse import bacc, mybir
from concourse.bass import ds
from concourse.bass_utils import run_bass_kernel_spmd
from concourse.masks import make_identity

B, N, NF, ND, NH = 16, 128, 64, 100, 64
NCORES = 8
BPC = B // NCORES  # molecules per core

F32 = mybir.dt.float32
BF16 = mybir.dt.bfloat16

G = 4  # i's per partition half per group; PSUM free dim = G*N = 512
NG = N // (2 * G)  # 16 groups per molecule
LW = 32  # i's per DMA window per partition half
NTW = LW // G  # groups per window
NW = (N // 2) // LW  # windows per molecule



def _emit(tc):
    nc = tc.nc
    dist = nc.dram_tensor("dist", (BPC, N, N, ND), F32, kind="ExternalInput").ap()
    atom = nc.dram_tensor("atom", (BPC, N, NF), F32, kind="ExternalInput").ap()
    w_cf = nc.dram_tensor("w_cf", (NF, NH), F32, kind="ExternalInput").ap()
    w_df = nc.dram_tensor("w_df", (ND, NH), F32, kind="ExternalInput").ap()
    w_fc = nc.dram_tensor("w_fc", (NH, NF), F32, kind="ExternalInput").ap()
    b_cf = nc.dram_tensor("b_cf", (1, NH), F32, kind="ExternalInput").ap()
    b_df = nc.dram_tensor("b_df", (1, NH), F32, kind="ExternalInput").ap()
    out = nc.dram_tensor("out", (BPC, N, NF), F32, kind="ExternalOutput").ap()

    mult = mybir.AluOpType.mult
    add = mybir.AluOpType.add

    with (
        tc.tile_pool(name="consts", bufs=1) as consts,
        tc.tile_pool(name="dloads", bufs=4) as dloads,
        tc.tile_pool(name="dbf", bufs=1) as dbf,
        tc.tile_pool(name="dtv", bufs=6) as dtv,
        tc.tile_pool(name="work", bufs=4) as work,
        tc.tile_pool(name="perb", bufs=2) as perb,
        tc.tile_pool(name="ptp", bufs=3, space="PSUM") as ptp,
        tc.tile_pool(name="pmm1", bufs=2, space="PSUM") as pmm1,
        tc.tile_pool(name="pmm2", bufs=2, space="PSUM") as pmm2,
        tc.tile_pool(name="psmall", bufs=1, space="PSUM") as psmall,
    ):
        # bf16 identity first: it gates the dist transposes.  The fp32
        # identity (finalize only) is built after the window descriptor gens
        # so it doesn't delay them on the gpsimd queue.
        identity_bf = consts.tile([128, 128], BF16)
        make_identity(nc, identity_bf)
        ones_f32 = consts.tile([1, N], F32)
        nc.vector.memset(ones_f32, 1.0)

        # --- dist windows: SWDGE fp32->bf16 cast-on-wire loads.
        # Window (b, w) holds i in [u*64 + w*LW, ...+LW) for both halves u;
        # all four windows stay resident (51.2KB/partition).
        d_tiles = {}
        for b in range(BPC):
            for w in range(NW):
                d_tiles[(b, w)] = dbf.tile(
                    [N, 2, LW, ND], BF16, tag=f"dbf_{b}_{w}", name=f"dbf_{b}_{w}"
                )

        def load_window(b, w, npiece):
            d_b = d_tiles[(b, w)]
            for p in range(npiece):
                for u in range(2):
                    nc.gpsimd.dma_start(
                        d_b[:, u, ds(p * (LW // npiece), LW // npiece)],
                        dist[
                            b,
                            ds(
                                u * (N // 2) + LW * w + p * (LW // npiece),
                                LW // npiece,
                            ),
                        ].rearrange("i j d -> j i d"),
                    )

        # All window descriptor gens up front, in consumption order — gpsimd
        # has no other steady-state work, and the DMA queues then stay ahead
        # of the pipeline with no mid-kernel window stalls.  The cold-start
        # window is split finer so the first transposes start early.
        load_window(0, 0, npiece=4)
        load_window(0, 1, npiece=2)
        load_window(1, 0, npiece=2)
        load_window(1, 1, npiece=2)

        # Atom features first on the scalar ring (they gate the atom_h prep);
        # weights ride the otherwise-idle sync ring, broadcast-duplicated in
        # a single DMA where a doubled copy is needed.
        atom_ins = []
        for b in range(BPC):
            atom_in = dloads.tile([N, NF], F32, tag="atom_in", bufs=2, name=f"atom_in{b}")
            nc.scalar.dma_start(atom_in, atom[b])
            atom_ins.append(atom_in)

        w_df_f = consts.tile([ND, NH], F32)
        nc.sync.dma_start(w_df_f, w_df)
        w_df_bf = consts.tile([ND, NH], BF16)
        nc.vector.tensor_copy(w_df_bf, w_df_f)

        # W_fc stacked twice vertically so the partition-hi mm2 has its
        # stationary at the same base partition as its rhs.
        w_fc_f = consts.tile([2 * NH, NF], F32)
        nc.sync.dma_start(w_fc_f[:NH], w_fc)
        nc.sync.dma_start(w_fc_f[NH:], w_fc)
        w_fc_bf = consts.tile([2 * NH, NF], BF16)
        nc.vector.tensor_copy(w_fc_bf, w_fc_f)

        # W_cf (and b_cf) duplicated horizontally so atom_hT comes out
        # stacked twice vertically: [2*NH, N] for the partition-packed gate.
        # bf16 so the atom prep can use the bf16 identity (the fp32 identity
        # is built late).
        w_cf_dup = consts.tile([NF, 2 * NH], F32)
        nc.sync.dma_start(
            w_cf_dup.rearrange("f (two h) -> f two h", two=2),
            w_cf[:, None, :].to_broadcast((NF, 2, NH)),
        )
        w_cf_bf = consts.tile([NF, 2 * NH], BF16)
        nc.vector.tensor_copy(w_cf_bf, w_cf_dup)
        bcf_dup = consts.tile([1, 2 * NH], F32)
        nc.sync.dma_start(
            bcf_dup.rearrange("o (two h) -> o two h", two=2),
            b_cf[:, None, :].to_broadcast((1, 2, NH)),
        )
        bcf_bf = consts.tile([1, 2 * NH], BF16)
        nc.vector.tensor_copy(bcf_bf, bcf_dup)
        ones_bf = consts.tile([1, N], BF16)
        nc.vector.memset(ones_bf, 1.0)

        # Preload the tanh table set (~1.3us) while the first DMAs are in flight.
        warm_tanh = consts.tile([1, 8], F32)
        nc.scalar.activation(warm_tanh, ones_f32[:, :8], mybir.ActivationFunctionType.Tanh)

        # b_df as a per-partition column [2*NH, 1] for the fused gate
        # (out1 + b_df) * atom_h^T.  Built by a K=1 transpose-matmul.
        bdf_dup = consts.tile([1, 2 * NH], F32)
        nc.sync.dma_start(
            bdf_dup.rearrange("o (two h) -> o two h", two=2),
            b_df[:, None, :].to_broadcast((1, 2, NH)),
        )

        bdf_col_ps = psmall.tile([2 * NH, 1], F32, tag="small_ps")
        nc.tensor.matmul(bdf_col_ps, bdf_dup, ones_f32[:, :1], start=True, stop=True)
        bdf_col = consts.tile([2 * NH, 1], F32)
        nc.vector.tensor_copy(bdf_col, bdf_col_ps)

        # --- per-molecule prep: atom_hT2[(u h), j] = (atom[b]@W_cf+b_cf)^T x2
        # (bf16 throughout so only identity_bf is needed this early)
        atom_hT2s = []
        for b in range(BPC):
            atom_bf = work.tile([N, NF], BF16, tag="atom_bf", bufs=1)
            nc.vector.tensor_copy(atom_bf, atom_ins[b])
            atomT_ps = psmall.tile([NF, N], BF16, tag="small_ps")
            nc.tensor.transpose(atomT_ps, atom_bf, identity_bf)
            atomT = work.tile([NF, N], BF16, tag="atomT", bufs=1)
            nc.vector.tensor_copy(atomT, atomT_ps)
            ah_ps = psmall.tile([2 * NH, N], F32, tag="small_ps")
            nc.tensor.matmul(ah_ps, w_cf_bf, atomT, start=True, stop=False)
            nc.tensor.matmul(ah_ps, bcf_bf, ones_bf, start=False, stop=True)
            atom_hT2 = perb.tile([2 * NH, N], BF16, tag="atom_hT2")
            nc.vector.tensor_copy(atom_hT2, ah_ps)
            atom_hT2s.append(atom_hT2)

        # --- main loop: 3-stage software pipeline over the 32 groups so the
        # in-order PE stream is [T(g), mm1(g-1), mm2(g-2), T(g+1), ...]: the
        # PSUM->SBUF copy of group g then has a full period of latency slack
        # before mm1(g) consumes it, and the gate likewise before mm2.
        groups = [(b, t) for b in range(BPC) for t in range(NG)]
        res_packs = {}
        st_dT = {}  # gi -> dT tile (front -> mid)
        st_gated = {}  # gi -> gatedT tile (mid -> back)
        st_out2 = {}  # gi -> out2_ps tile (back-pe -> back-act)

        def finalize(b):
            # out[b] = res_pack^T + atom[b] (per half)
            res_pack = res_packs[b]
            for u in range(2):
                resT_ps = psmall.tile([N // 2, NF], BF16, tag="small_ps")
                nc.tensor.transpose(
                    resT_ps,
                    res_pack[ds(u * NF, NF)],
                    identity_bf[ds(u * NF, NF), ds(u * NF, N // 2)],
                )
                out_sb = work.tile([N // 2, NF], F32, tag="out_sb", bufs=2)
                nc.vector.tensor_add(
                    out_sb, resT_ps, atom_ins[b][ds(u * (N // 2), N // 2)]
                )
                # sync ring: the scalar sequencer is busy with copy/tanh
                # dispatch at finalize time
                nc.sync.dma_start(out[b, ds(u * (N // 2), N // 2)], out_sb)

        for gi in range(len(groups) + 2):
            # ---- front(gi): window prefetch, transposes, PSUM->SBUF copy
            if gi < len(groups):
                b, t = groups[gi]
                if t == 0:
                    res_packs[b] = perb.tile(
                        [2 * NF, G * NG], BF16, tag="res_pack", name=f"res_pack{b}"
                    )
                d_b = d_tiles[(b, t // NTW)]
                tt = t % NTW
                tp_ps = ptp.tile([ND, 2 * G * N], BF16, tag="tp")
                for u in range(2):
                    for q in range(G):
                        nc.tensor.transpose(
                            tp_ps[:, ds((u * G + q) * N, N)],
                            d_b[:, u, tt * G + q, :ND],
                            identity_bf,
                        )
                # PSUM->SBUF drain split across scalar and DVE (gpsimd cannot
                # read PSUM)
                dT = dtv.tile([ND, 2 * G * N], BF16, tag="dT")
                nc.scalar.copy(dT[:, :768], tp_ps[:, :768])
                nc.vector.tensor_copy(dT[:, 768:], tp_ps[:, 768:])
                st_dT[gi] = dT

            # ---- back(gi-2): mm2 pair, tanh, j-reduce
            gb = gi - 2
            if gb >= 0:
                bb, tb = groups[gb]
                gatedT = st_gated.pop(gb)
                out2_ps = pmm2.tile([2 * NF, G * N], F32, tag="out2")
                nc.tensor.matmul(
                    out2_ps[:NF], w_fc_bf[:NH], gatedT[:NH], start=True, stop=True
                )
                nc.tensor.matmul(
                    out2_ps[NF:], w_fc_bf[NH:], gatedT[NH:], start=True, stop=True
                )
                tanh_sb = work.tile([2 * NF, G * N], BF16, tag="tanh_sb")
                nc.scalar.activation(
                    tanh_sb, out2_ps, mybir.ActivationFunctionType.Tanh
                )
                with nc.allow_low_precision(
                    reason="j-sums of O(100) tanh values; bf16 output "
                    "rounding adds ~0.4% which the 2e-2 budget absorbs"
                ):
                    nc.vector.tensor_reduce(
                        res_packs[bb][:, ds(G * tb, G)],
                        tanh_sb.rearrange("f (i j) -> f i j", i=G),
                        axis=mybir.AxisListType.X,
                        op=add,
                    )
                if tb == NG - 1:
                    finalize(bb)
            # ---- mid(gi-1): mm1 pair, gate
            gm = gi - 1
            if 0 <= gm < len(groups):
                bm, tm = groups[gm]
                dT = st_dT.pop(gm)
                out1_ps = pmm1.tile([2 * NH, G * N], F32, tag="out1")
                nc.tensor.matmul(
                    out1_ps[:NH], w_df_bf, dT[:, : G * N], start=True, stop=True
                )
                nc.tensor.matmul(
                    out1_ps[NH:], w_df_bf, dT[:, G * N :], start=True, stop=True
                )
                gatedT = work.tile([2 * NH, G * N], BF16, tag="gatedT")
                nc.vector.scalar_tensor_tensor(
                    gatedT.rearrange("h (i j) -> h i j", i=G),
                    out1_ps.rearrange("h (i j) -> h i j", i=G),
                    bdf_col,
                    atom_hT2s[bm][:, None, :].to_broadcast((2 * NH, G, N)),
                    add,
                    mult,
                )
                st_gated[gm] = gatedT



_NC_CACHE = None


def _get_nc():
    global _NC_CACHE
    if _NC_CACHE is None:
        nc = bacc.Bacc("TRN2", target_bir_lowering=False, debug=False)
        with tile.TileContext(nc) as tc:
            _emit(tc)
        nc.compile()
        _NC_CACHE = nc
    return _NC_CACHE


def _numpy_reference(atom, dist, mask, w_cf, w_df, w_fc, b_cf, b_df):
    dist_h = np.einsum("bijd,dh->bijh", dist, w_df) + b_df
    atom_h = np.einsum("bjf,fh->bjh", atom, w_cf) + b_cf
    gated = dist_h * atom_h[:, None, :, :]
    o = np.einsum("bijh,hf->bijf", gated, w_fc)
    o = np.tanh(o * mask[..., None])
    return (o.sum(axis=2) + atom).astype(np.float32)


def run_sharded(inputs, trace=False):
    """Shard over the batch axis, run on 8 cores, gather. Returns (out, results)."""
    atom = np.ascontiguousarray(np.asarray(inputs["atom_features"], np.float32))
    dist = np.ascontiguousarray(np.asarray(inputs["distance_matrix"], np.float32))
    w_cf = np.ascontiguousarray(np.asarray(inputs["W_cf"], np.float32))
    w_df = np.ascontiguousarray(np.asarray(inputs["W_df"], np.float32))
    w_fc = np.ascontiguousarray(np.asarray(inputs["W_fc"], np.float32))
    b_cf = np.asarray(inputs["b_cf"], np.float32).reshape(1, NH)
    b_df = np.asarray(inputs["b_df"], np.float32).reshape(1, NH)

    nc = _get_nc()
    in_maps = []
    for c in range(NCORES):
        sl = slice(c * BPC, (c + 1) * BPC)
        in_maps.append(
            {
                "dist": dist[sl],
                "atom": atom[sl],
                "w_cf": w_cf,
                "w_df": w_df,
                "w_fc": w_fc,
                "b_cf": b_cf,
                "b_df": b_df,
            }
        )
    res = run_bass_kernel_spmd(nc, in_maps, core_ids=list(range(NCORES)), trace=trace)
    out = np.concatenate([res.results[c]["out"] for c in range(NCORES)], axis=0)
    return out, res


def kernel(**inputs) -> np.ndarray:
    mask = np.asarray(inputs["distance_matrix_mask"], np.float32)
    if not np.all(mask == 1.0):
        # The hardware pipeline folds the (always-ones) mask away; keep a
        # correct path for arbitrary masks.
        return _numpy_reference(
            np.asarray(inputs["atom_features"], np.float32),
            np.asarray(inputs["distance_matrix"], np.float32),
            mask,
            np.asarray(inputs["W_cf"], np.float32),
            np.asarray(inputs["W_df"], np.float32),
            np.asarray(inputs["W_fc"], np.float32),
            np.asarray(inputs["b_cf"], np.float32),
            np.asarray(inputs["b_df"], np.float32),
        )
    out, _ = run_sharded(inputs)
    return out
